# revision 10
# baseline (speedup 1.0000x reference)
"""Multi-head-free attention (softmax over the QUERY axis) on 8 trn2 NeuronCores.

Problem: x:[4,2048,1024], Wq/Wk/Wv:[1024,1024], bq/bk/bv:[1024]
    q = x@Wq+bq ; k = x@Wk+bk ; v = x@Wv+bv
    scores = einsum('bqd,bkd->bqk', q, k) / 32
    attn   = softmax(scores, axis=1)          # over q (dim 1)!
    out    = einsum('bqk,bkv->bqv', attn, v)

Sharding: 4 batches x 2-way split of the KEY axis across 8 cores
(core c -> batch c//2, key-half c%2).  Because softmax normalizes over
q for each fixed k, a k-split keeps the softmax fully local per core.
Each core computes a partial out[q, dv] summed over its k-half; a
2-core ReduceScatter (over the q axis) completes the sum, and rank r of
each pair returns q-rows [r*1024, (r+1)*1024) of its batch.

All matmuls run as float32r (full PE rate at N=512 moving dim, fp32
storage).  The attn*V contraction runs in bf16 (attn weights + V), with
fp32 PSUM accumulation.
"""

import sys

if "/opt/trn_rl_repo" not in sys.path:
    sys.path.insert(0, "/opt/trn_rl_repo")

import numpy as np

P = 128  # SBUF partitions


class Cfg:
    def __init__(self, B=4, S=2048, E=1024, D=1024, NB=512, n_cores=8, mm="f32r"):
        self.B, self.S, self.E, self.D, self.NB = B, S, E, D, NB
        self.mm = mm
        self.SH = S // 2          # per-core key-half length
        self.NE = E // P          # e (contraction) tiles
        self.ND = D // P          # d tiles
        self.NQB = S // NB        # q 512-blocks (full)
        self.NKB = self.SH // NB  # k 512-blocks (half)
        self.NKT = self.SH // P   # k 128-tiles (half)
        self.NQT = S // P         # q 128-tiles (full)
        self.NDVB = D // NB       # dv 512-blocks
        self.n_cores = n_cores
        self.groups = [[2 * i, 2 * i + 1] for i in range(n_cores // 2)]


PROD = Cfg()


def build_nc(cfg: Cfg):
    from concourse import bacc, bass, mybir, tile

    f32 = mybir.dt.float32
    f32r = mybir.dt.float32r
    bf16 = mybir.dt.bfloat16
    AF = mybir.ActivationFunctionType
    X = mybir.AxisListType.X
    ts = bass.ts

    B, S, E, D, NB = cfg.B, cfg.S, cfg.E, cfg.D, cfg.NB
    SH, NE, ND = cfg.SH, cfg.NE, cfg.ND
    NQB, NKB, NKT, NQT, NDVB = cfg.NQB, cfg.NKB, cfg.NKT, cfg.NQT, cfg.NDVB
    inv_sqrt_d = 1.0 / float(np.sqrt(np.float32(D)))

    nc = bacc.Bacc(None, num_devices=cfg.n_cores)
    dt_in = bf16 if cfg.mm == "bf16" else f32r

    # Per-core inputs (host pre-shards / pre-transposes).
    xt_d = nc.declare_dram_parameter("xt", [E, S], dt_in, isOutput=False)
    xth_d = nc.declare_dram_parameter("xth", [E, SH], dt_in, isOutput=False)
    wq_d = nc.declare_dram_parameter("wq", [E, D], dt_in, isOutput=False)
    wk_d = nc.declare_dram_parameter("wk", [E, D], dt_in, isOutput=False)
    wv_d = nc.declare_dram_parameter("wv", [E, D], dt_in, isOutput=False)
    bq_d = nc.declare_dram_parameter("bq", [D, 1], f32, isOutput=False)
    bk_d = nc.declare_dram_parameter("bk", [D, 1], f32, isOutput=False)
    bv_d = nc.declare_dram_parameter("bv", [1, D], dt_in, isOutput=False)
    ones_d = nc.declare_dram_parameter("onesv", [1, P], dt_in, isOutput=False)
    y_d = nc.declare_dram_parameter("y", [SH, D], f32, isOutput=True)

    def r(ap):  # tensors feeding the PE are already dt_in (f32r or bf16)
        return ap

    with tile.TileContext(nc) as tc:
        with (
            tc.tile_pool(name="w", bufs=NE + 1) as w_pool,
            tc.tile_pool(name="x", bufs=min(12, 2 * NE + 2)) as x_pool,
            tc.tile_pool(name="kt", bufs=1) as kt_pool,
            tc.tile_pool(name="qt", bufs=2 * ND) as qt_pool,
            tc.tile_pool(name="pt", bufs=1) as pt_pool,
            tc.tile_pool(name="v", bufs=1) as v_pool,
            tc.tile_pool(name="ob", bufs=6) as out_pool,
            tc.tile_pool(name="small", bufs=1) as small_pool,
            tc.tile_pool(name="ps", bufs=8, space="PSUM") as ps_pool,
            tc.tile_pool(name="dram", bufs=1, space="DRAM") as dram_pool,
        ):
            part_d = dram_pool.tile([S, D], f32)
            rsout_d = dram_pool.tile([SH, D], f32)

            # ---- constants ----
            ones_t = small_pool.tile([1, P], dt_in, tag="ones")
            nc.sync.dma_start(ones_t[:], ones_d[:])
            bv_t = small_pool.tile([1, D], dt_in, tag="bvrow")
            nc.sync.dma_start(bv_t[:], bv_d[:])
            bq_t, bk_t = [], []
            for dt in range(ND):
                bqt = small_pool.tile([P, 1], f32, tag=f"bq{dt}")
                nc.sync.dma_start(bqt[:], bq_d[ts(dt, P), :])
                bq_t.append(bqt)
                bkt = small_pool.tile([P, 1], f32, tag=f"bk{dt}")
                nc.sync.dma_start(bkt[:], bk_d[ts(dt, P), :])
                bk_t.append(bkt)

            # ---- phase K: KT[dt][d_in_tile, k] = K[k, d]  (k = my half) ----
            kt_tiles = []
            for dt in range(ND):
                ktt = kt_pool.tile([P, SH], dt_in, tag=f"kt{dt}", name=f"ktt{dt}")
                kt_tiles.append(ktt)
            wk_t = []
            for et in range(NE):
                w = w_pool.tile([P, D], dt_in, tag="w", name=f"wk{et}")
                nc.sync.dma_start(w[:], wk_d[ts(et, P), :])
                wk_t.append(w)
            for kb in range(NKB):
                xk = []
                for et in range(NE):
                    t = x_pool.tile([P, NB], dt_in, tag="x", name=f"xk{kb}_{et}")
                    nc.sync.dma_start(t[:], xth_d[ts(et, P), ts(kb, NB)])
                    xk.append(t)
                for dt in range(ND):
                    ps = ps_pool.tile([P, NB], f32, tag="ps", name="psk")
                    for et in range(NE):
                        nc.tensor.matmul(
                            ps[:], r(wk_t[et][:, ts(dt, P)]), r(xk[et][:]),
                            start=(et == 0), stop=(et == NE - 1),
                        )
                    nc.scalar.activation(
                        kt_tiles[dt][:, ts(kb, NB)], ps[:], AF.Identity, bias=bk_t[dt][:]
                    )

            # ---- phase Q+S fused: per q-block project Q, then scores+exp ----
            pt_tiles = []
            rs_t = []
            for kt in range(NKT):
                ptt = pt_pool.tile([P, S], bf16, tag=f"pt{kt}", name=f"ptt{kt}")
                pt_tiles.append(ptt)
                rst = small_pool.tile([P, NQB], f32, tag=f"rs{kt}", name=f"rst{kt}")
                rs_t.append(rst)
            wq_t = []
            for et in range(NE):
                w = w_pool.tile([P, D], dt_in, tag="w", name=f"wq{et}")
                nc.sync.dma_start(w[:], wq_d[ts(et, P), :])
                wq_t.append(w)
            for qb in range(NQB):
                xq = []
                for et in range(NE):
                    t = x_pool.tile([P, NB], dt_in, tag="x", name=f"xq{qb}_{et}")
                    nc.sync.dma_start(t[:], xt_d[ts(et, P), ts(qb, NB)])
                    xq.append(t)
                qt_t = []
                for dt in range(ND):
                    ps = ps_pool.tile([P, NB], f32, tag="ps", name="psq")
                    for et in range(NE):
                        nc.tensor.matmul(
                            ps[:], r(wq_t[et][:, ts(dt, P)]), r(xq[et][:]),
                            start=(et == 0), stop=(et == NE - 1),
                        )
                    q = qt_pool.tile([P, NB], dt_in, tag="qt", name=f"qtt{qb}_{dt}")
                    nc.scalar.activation(q[:], ps[:], AF.Identity, bias=bq_t[dt][:])
                    qt_t.append(q)
                for kt in range(NKT):
                    ps = ps_pool.tile([P, NB], f32, tag="ps", name="pss")
                    for dt in range(ND):
                        nc.tensor.matmul(
                            ps[:], r(kt_tiles[dt][:, ts(kt, P)]), r(qt_t[dt][:]),
                            start=(dt == 0), stop=(dt == ND - 1),
                        )
                    # PT = exp(scores/sqrt(D)); row-sum (over q) accumulated
                    nc.scalar.activation(
                        pt_tiles[kt][:, ts(qb, NB)], ps[:], AF.Exp,
                        scale=inv_sqrt_d,
                        accum_out=rs_t[kt][:, qb:qb + 1],
                    )

            # softmax denominators 1/D[k] (fully local: full q range on-core)
            rcp_t = []
            for kt in range(NKT):
                rsum = small_pool.tile([P, 1], f32, tag=f"rsum{kt}", name=f"rsum{kt}")
                nc.vector.reduce_sum(rsum[:], rs_t[kt][:], axis=X)
                rcp = small_pool.tile([P, 1], f32, tag=f"rcp{kt}", name=f"rcp{kt}")
                nc.vector.reciprocal(rcp[:], rsum[:])
                rcp_t.append(rcp)

            # ---- phase V: V[k, dv] = (X@Wv + bv) * (1/D[k]) ----
            v_tiles = []
            for kt in range(NKT):
                vt = v_pool.tile([P, D], bf16, tag=f"v{kt}", name=f"vt{kt}")
                v_tiles.append(vt)
            wv_t = []
            for et in range(NE):
                w = w_pool.tile([P, D], dt_in, tag="w", name=f"wv{et}")
                nc.sync.dma_start(w[:], wv_d[ts(et, P), :])
                wv_t.append(w)
            KT_PER_B = NB // P  # k-tiles per 512-block
            for kh in range(NKB):
                xv = []
                for et in range(NE):
                    t = x_pool.tile([P, NB], dt_in, tag="x", name=f"xv{kh}_{et}")
                    nc.sync.dma_start(t[:], xth_d[ts(et, P), ts(kh, NB)])
                    xv.append(t)
                for kt4 in range(KT_PER_B):
                    kt = kh * KT_PER_B + kt4
                    for dvb in range(NDVB):
                        ps = ps_pool.tile([P, NB], f32, tag="ps", name="psv")
                        for et in range(NE):
                            nc.tensor.matmul(
                                ps[:], r(xv[et][:, ts(kt4, P)]),
                                r(wv_t[et][:, ts(dvb, NB)]),
                                start=(et == 0), stop=False,
                            )
                        # += 1^T @ bv  (broadcasts bv along k rows)
                        nc.tensor.matmul(
                            ps[:], r(ones_t[:]), r(bv_t[:, ts(dvb, NB)]),
                            start=False, stop=True,
                        )
                        nc.vector.tensor_scalar_mul(
                            v_tiles[kt][:, ts(dvb, NB)], ps[:], rcp_t[kt][:]
                        )

            # ---- phase AV: part[q, dv] = sum_{k in my half} PT[k,q] * V[k,dv] ----
            for qt in range(NQT):
                for dvb in range(NDVB):
                    ps = ps_pool.tile([P, NB], f32, tag="ps", name="psav")
                    for kt in range(NKT):
                        nc.tensor.matmul(
                            ps[:], pt_tiles[kt][:, ts(qt, P)],
                            v_tiles[kt][:, ts(dvb, NB)],
                            start=(kt == 0), stop=(kt == NKT - 1),
                        )
                    ob = out_pool.tile([P, NB], f32, tag="ob", name="ob")
                    nc.scalar.copy(ob[:], ps[:])
                    nc.sync.dma_start(part_d[ts(qt, P), ts(dvb, NB)], ob[:])

            # ---- pairwise ReduceScatter over q, then emit my q-rows ----
            nc.gpsimd.collective_compute(
                "ReduceScatter",
                mybir.AluOpType.add,
                replica_groups=cfg.groups,
                ins=[part_d[:].opt()],
                outs=[rsout_d[:].opt()],
            )
            nc.sync.dma_start(y_d[:], rsout_d[:])

    nc.compile()
    return nc


def make_in_maps(cfg: Cfg, x, Wq, bq, Wk, bk, Wv, bv):
    SH = cfg.SH
    f32 = np.float32
    if cfg.mm == "bf16":
        import ml_dtypes
        dt_in = ml_dtypes.bfloat16
    else:
        dt_in = f32
    in_maps = []
    shared = {
        "wq": np.ascontiguousarray(Wq, dtype=dt_in),
        "wk": np.ascontiguousarray(Wk, dtype=dt_in),
        "wv": np.ascontiguousarray(Wv, dtype=dt_in),
        "bq": np.ascontiguousarray(np.reshape(bq, (-1, 1)), dtype=f32),
        "bk": np.ascontiguousarray(np.reshape(bk, (-1, 1)), dtype=f32),
        "bv": np.ascontiguousarray(np.reshape(bv, (1, -1)), dtype=dt_in),
        "onesv": np.ones((1, 128), dtype=dt_in),
    }
    for c in range(cfg.n_cores):
        b, h = c // 2, c % 2
        xb = np.asarray(x[b], dtype=f32)
        m = dict(shared)
        m["xt"] = np.ascontiguousarray(xb.T, dtype=dt_in)
        m["xth"] = np.ascontiguousarray(xb[h * SH:(h + 1) * SH, :].T, dtype=dt_in)
        in_maps.append(m)
    return in_maps


def run(inputs: dict, cfg: Cfg = PROD, trace: bool = False):
    from concourse.bass_utils import run_bass_kernel_spmd

    nc = build_nc(cfg)
    in_maps = make_in_maps(cfg, inputs["x"], inputs["Wq"], inputs["bq"],
                           inputs["Wk"], inputs["bk"], inputs["Wv"], inputs["bv"])
    res = run_bass_kernel_spmd(nc, in_maps, list(range(cfg.n_cores)), trace=trace)
    B, S, D = cfg.B, cfg.S, cfg.D
    out = np.empty((B, S, D), dtype=np.float32)
    for b in range(B):
        out[b, : cfg.SH] = res.results[2 * b]["y"]
        out[b, cfg.SH:] = res.results[2 * b + 1]["y"]
    return out, res


def kernel(**inputs) -> np.ndarray:
    out, _ = run(inputs, PROD, trace=False)
    return out
